# revision 1
# baseline (speedup 1.0000x reference)
"""Trainium2 Bass kernel for a CPPN-style dense MLP forward pass.

Network (per pixel): 11 -> [32 x 23 tanh layers] -> 3 sigmoid.
  h = tanh(x @ W1.T); 22x: h = tanh(h @ Whid[l].T); out = sigmoid(h @ Wout.T)

Full inputs:  x [4194304, 11] f32, W1 [32, 11], Whid [22, 32, 32], Wout [3, 32]
Full output:  [4194304, 3] f32

Strategy: pure data parallel over 8 NeuronCores (pixels split 8 ways,
weights replicated).  Per core the kernel is ScalarE(tanh)-throughput
bound, so the layout keeps ACT ~100% busy on large [128, 2048]
activations while the PE runs the 32x32 matmuls 16-at-a-time via
tile_position packing (all 16 32x32 sub-arrays concurrently).

Layout per core: pixels processed in "supertiles" of 16 tiles x 512
pixels = 8192 pixels.  Activations live feature-major: tile (a,b) holds
[32 features, 512 pixels] at SBUF partitions [32u:32u+32], free offset
512*v, where (u,v)=(a,b) on even layers and (b,a) on odd layers.  Each
layer = 16 concurrent matmuls at tile_position (32u, 32v) writing one
[128, 2048] PSUM half (4 banks), then one big ACT tanh PSUM->SBUF.
Two supertile streams are interleaved (PSUM ping-pong) so the PE fills
one PSUM half while ACT drains the other.

I/O avoids small-packet DMA death: x is loaded pixel-major with 44B
contiguous chunks and block-transposed to feature-major on the (idle)
VectorE via its 32x32 STREAM_TRANSPOSE; the sigmoid output is
block-transposed back so the store scatters 12B/pixel chunks with a
32-row outer dim (spreads across all 16 DMA engines).  All DMAs are
issued from SyncE - DMA issue occupies the issuing engine's
instruction stream and must stay off the ACT critical path.

Matmuls are full fp32 (2-pass LOW/HIGH on the PE).  The 24-layer tanh
chain is chaotic (Lyapunov growth ~700x): fp32 implementations already
differ from each other by ~2e-4 L2 on the final output, and reduced
matmul precision (float32r, ~12 mantissa bits, would be 2x faster and
single-pass) amplifies to ~0.14 L2 - unusable.  Measured: ~3.20 ms on
hardware, vs a 2.95 ms ScalarE floor (1536 ACTIVATEs x (2048+222)cyc
@ 1.2 GHz); PE/DVE/DMA are all hidden under the tanh stream.
"""

import os
import sys

if "/opt/trn_rl_repo" not in sys.path:
    sys.path.insert(0, "/opt/trn_rl_repo")

import numpy as np

N_CORES = 8
N_PIX = 4194304
P_CORE = N_PIX // N_CORES      # 524288 pixels per core
D_IN = 11
D_H = 32
N_LAYERS = 24                  # 1 input + 22 hidden + 1 output
F = 512                        # pixels per tile (one PSUM bank of fp32)
ST_PIX = 16 * F                # 8192 pixels per supertile
N_ST = P_CORE // ST_PIX        # 64 supertiles per core
N_PAIRS = N_ST // 2            # 32 interleaved supertile pairs

_BUILD_CACHE = {}


def _build(n_pairs):
    """Build + bass-compile the per-core program. Returns the Bacc object."""
    import concourse.bass as bass  # noqa: F401
    import concourse.tile as tile
    from concourse import bacc, mybir
    from contextlib import ExitStack

    f32 = mybir.dt.float32
    Tanh = mybir.ActivationFunctionType.Tanh
    Sigmoid = mybir.ActivationFunctionType.Sigmoid

    nc = bacc.Bacc(
        "TRN2", target_bir_lowering=False, debug=False, num_devices=N_CORES
    )
    x_ap = nc.dram_tensor("x", [P_CORE, D_IN], f32, kind="ExternalInput").ap()
    w_ap = nc.dram_tensor("w", [128, N_LAYERS * 32], f32, kind="ExternalInput").ap()
    wbd_ap = nc.dram_tensor("wbd", [128, 22 * 128], f32, kind="ExternalInput").ap()
    o_ap = nc.dram_tensor("o", [P_CORE, 3], f32, kind="ExternalOutput").ap()

    with tile.TileContext(nc) as tc, ExitStack() as ctx:
        wp = ctx.enter_context(tc.tile_pool(name="wp", bufs=1))
        xrp = ctx.enter_context(tc.tile_pool(name="xrp", bufs=4))
        xp = ctx.enter_context(tc.tile_pool(name="xp", bufs=4))
        hp = ctx.enter_context(tc.tile_pool(name="hp", bufs=4))
        sp = ctx.enter_context(tc.tile_pool(name="sp", bufs=6))
        pp = ctx.enter_context(tc.tile_pool(name="pp", bufs=2, space="PSUM"))

        mm_dt = (
            mybir.dt.float32r
            if os.environ.get("BASSK_MMDT", "f32") == "f32r"
            else f32
        )

        Wf = wp.tile([128, N_LAYERS * 32], f32)
        nc.sync.dma_start(Wf[:], w_ap[:])
        if mm_dt != f32:
            # Hidden layers run as full-array [128,128] block-diagonal f32r
            # matmuls (f32r only supports column-group 0, so no 16-way
            # packing); the explicit scalar copy is the required f32r
            # rounding producer.
            Wbf = wp.tile([128, 22 * 128], f32)
            nc.sync.dma_start(Wbf[:], wbd_ap[:])
            Wbr = wp.tile([128, 22 * 128], mm_dt)
            nc.scalar.copy(Wbr[:], Wbf[:])

        def load_x(s, eng):
            # Stage 1: pixel-major load, 44B contiguous chunks per pixel row,
            # laid out so that 32x32 block-transpose yields feature-major
            # tiles: XR[32u+p, 32c+f] = x[s*8192 + u*2048 + 32c + p, f].
            XR = xrp.tile([128, 2048], f32)
            for u in range(4):
                p0 = s * ST_PIX + u * 2048
                eng.dma_start(
                    XR[32 * u : 32 * u + 32, :].rearrange(
                        "p (c f) -> p c f", c=64, f=32
                    )[:, :, 0:D_IN],
                    x_ap[p0 : p0 + 2048, :].rearrange("(c p) f -> p c f", c=64, p=32),
                )
            # Stage 2: DVE 32x32 block transpose -> X[32u+f, 32c+p].
            X = xp.tile([128, 2048], f32)
            nc.vector.transpose(X[:], XR[:])
            return X


        def layer(H, k):
            """One layer for one supertile: 16 packed matmuls + one ACT."""
            Kd = D_IN if k == 0 else 32
            last = k == N_LAYERS - 1
            P_ = pp.tile([128, 2048], f32)
            if mm_dt != f32 and 1 <= k <= 22:
                # Hidden layer: 4 block-diagonal full-array f32r matmuls,
                # layout-preserving: tile (a,g) stays at [32g, 512a].
                for a in range(4):
                    nc.tensor.matmul(
                        P_[:, 512 * a : 512 * a + 512],
                        lhsT=Wbr[:, 128 * (k - 1) : 128 * k],
                        rhs=H[:, 512 * a : 512 * a + 512],
                        start=True,
                        stop=True,
                        tile_position=(0, 0),
                    )
            else:
                # fp32 16-way tile-position packing; iterate so consecutive
                # matmuls land on different PE row groups (LDWEIGHTS only
                # pulls ahead of in-flight MMs when row_grp differs).
                ab = [(a, b) for b in range(4) for a in range(4)]
                if k % 2 == 1:
                    ab = [(a, b) for a in range(4) for b in range(4)]
                for a, b in ab:
                        u, v = (a, b) if k % 2 == 0 else (b, a)
                        nc.tensor.matmul(
                            P_[32 * v : 32 * v + 32, 512 * u : 512 * u + 512],
                            lhsT=Wf[32 * u : 32 * u + Kd, 32 * k : 32 * k + 32],
                            rhs=H[32 * u : 32 * u + Kd, 512 * v : 512 * v + 512],
                            start=True,
                            stop=True,
                            tile_position=(32 * u, 32 * v),
                        )
            if last:
                Hn = sp.tile([128, 2048], f32)
                nc.scalar.activation(Hn[:], P_[:], Sigmoid)
            else:
                # Layer 22's output feeds the fp32 16-way output layer.
                h_dt = f32 if k == N_LAYERS - 2 else mm_dt
                Hn = hp.tile([128, 2048], h_dt)
                nc.scalar.activation(Hn[:], P_[:], Tanh)
            return Hn

        def store_out(s, S, eng):
            # Block-transpose back to pixel-major so the scatter uses 12B
            # chunks with a 32-row outer dim (spreads across all DMA engines):
            # SR[32a+p, 32c+f] = S[32a+f, 32c+p] = out feature f of pixel
            # s*8192 + a*2048 + 32c + p.
            SR = sp.tile([128, 2048], f32)
            nc.vector.transpose(SR[:], S[:])
            for a in range(4):
                p0 = s * ST_PIX + a * 2048
                eng.dma_start(
                    o_ap[p0 : p0 + 2048, :].rearrange("(c p) f -> p c f", c=64, p=32),
                    SR[32 * a : 32 * a + 32, :].rearrange(
                        "p (c f) -> p c f", c=64, f=32
                    )[:, :, 0:3],
                )

        for pair in range(n_pairs):
            sA, sB = 2 * pair, 2 * pair + 1
            HA, HB = load_x(sA, nc.sync), load_x(sB, nc.sync)
            # Interleave the two streams layer-by-layer so the PSUM pool's
            # two slots ping-pong A/B and ACT never waits on the PE.
            for k in range(N_LAYERS):
                HA = layer(HA, k)
                HB = layer(HB, k)
            store_out(sA, HA, nc.sync)
            store_out(sB, HB, nc.sync)

    nc.compile()
    return nc


def _get_program(n_pairs):
    if n_pairs not in _BUILD_CACHE:
        _BUILD_CACHE[n_pairs] = _build(n_pairs)
    return _BUILD_CACHE[n_pairs]


def _pack_weights(W1, Whid, Wout):
    """[128, 24*32]: per partition-group u, column block l*32 holds W_l.T."""
    WT = np.zeros((N_LAYERS, 32, 32), np.float32)
    WT[0, :D_IN, :] = np.asarray(W1, np.float32).T
    WT[1:23] = np.transpose(np.asarray(Whid, np.float32), (0, 2, 1))
    WT[23, :, :3] = np.asarray(Wout, np.float32).T
    Wh = np.zeros((128, N_LAYERS * 32), np.float32)
    blocks = WT.transpose(1, 0, 2).reshape(32, N_LAYERS * 32)
    for u in range(4):
        Wh[32 * u : 32 * u + 32, :] = blocks
    Wbd = np.zeros((128, 22, 128), np.float32)
    for g in range(4):
        Wbd[32 * g : 32 * g + 32, :, 32 * g : 32 * g + 32] = WT[1:23].transpose(
            1, 0, 2
        )
    return Wh, Wbd.reshape(128, 22 * 128)


def _run(x, W1, Whid, Wout, trace=False, n_pairs=None, **spmd_kwargs):
    from concourse.bass_utils import run_bass_kernel_spmd

    if n_pairs is None:
        n_pairs = int(os.environ.get("BASSK_PAIRS", N_PAIRS))
    nc = _get_program(n_pairs)

    x = np.ascontiguousarray(np.asarray(x, np.float32))
    assert x.shape == (N_PIX, D_IN), x.shape
    Wh, Wbd = _pack_weights(W1, Whid, Wout)

    in_maps = [
        {"x": x[i * P_CORE : (i + 1) * P_CORE], "w": Wh, "wbd": Wbd}
        for i in range(N_CORES)
    ]
    res = run_bass_kernel_spmd(
        nc, in_maps, list(range(N_CORES)), trace=trace, **spmd_kwargs
    )
    out = np.concatenate([res.results[i]["o"] for i in range(N_CORES)], axis=0)
    return out, res


def kernel(x, W1, Whid, Wout):
    out, _ = _run(x, W1, Whid, Wout)
    return out

